# revision 1
# baseline (speedup 1.0000x reference)
"""Trainium2 Bass kernel for nn_ChatbotModel (seq2seq GRU encoder/decoder with
Bahdanau attention and a 128->50257 vocab projection).

Strategy (8 NeuronCores):
  - The recurrent encoder/decoder state math is tiny ([64,128] states) and
    latency-bound, so it is REPLICATED on every core.
  - The vocab projection (dominant memory traffic: 257MB of logits) is sharded
    over the vocab dimension: each core computes logits for its ~6283-column
    slice of W_proj/b_proj and writes a [1280, 6656(bf16-padded)] output shard.
  - Host does only data movement: embedding gathers, masks, weight slicing,
    constant selector matrices, and final concat/cast.

Device-side formulation (transposed-state chain, states stored as [H, B]):
  - sigmoid(x) emulated as 0.5 + 0.5*tanh(0.5*x) so the whole kernel uses a
    single ACT table set (tanh + exp).
  - Encoder sequence-length copy-through implemented by adding +100 to the
    u-gate preactivation at invalid steps (u -> 1 => h' = h).
  - Attention scores/context use constant selector matmuls to batch the
    per-batch-row contractions on the PE array.
"""

import os
import sys

for _p in ("/opt/trn_rl_repo", "/root/.axon_site/_ro/trn_rl_repo"):
    if os.path.isdir(_p) and _p not in sys.path:
        sys.path.insert(0, _p)

import numpy as np

import concourse.bacc as bacc
import concourse.bass as bass
import concourse.mybir as mybir
import concourse.tile as tile
from concourse.bass_utils import run_bass_kernel_spmd

B, S, H, V, E = 64, 20, 128, 50257, 300
NCORES = 8
VSH = 6283            # ceil(V / 8); last core's real width is 6276
VC = 6656             # 13 * 512, padded per-core vocab width
SB = S * B            # 1280
BIG = 100.0
NEG = -1e30

F32 = mybir.dt.float32
BF16 = mybir.dt.bfloat16
NPBF16 = mybir.dt.np(BF16)
AF = mybir.ActivationFunctionType
OP = mybir.AluOpType
AX = mybir.AxisListType

LAST_RESULT = None    # BassKernelResults of the most recent run (for test.py)

_COL_CHUNKS = ((0, 512), (512, 512), (1024, 256))   # 1280 split for psum banks


def _build_nc():
    nc = bacc.Bacc("TRN2", target_bir_lowering=False, debug=False,
                   num_devices=NCORES)
    d = {}

    def din(name, shape, dt=F32):
        d[name] = nc.dram_tensor(name, list(shape), dt, kind="ExternalInput").ap()

    # embeddings (pre-gathered, transposed to [E, (t,b)]) in 128-row chunks
    for nm in ("ex", "ey"):
        din(f"{nm}0", (128, SB)); din(f"{nm}1", (128, SB)); din(f"{nm}2", (44, SB))
    # x-part weight chunks (lhsT, contraction rows match ex chunks)
    for nm, g in (("e0gx", 256), ("e0cx", 128), ("d0gx", 256), ("d0cx", 128)):
        din(f"{nm}0", (128, g)); din(f"{nm}1", (128, g)); din(f"{nm}2", (44, g))
    # recurrent weights
    din("wg0h", (128, 256)); din("wc0h", (128, 128))       # e0 (wc0h pre-halved)
    din("wg1t", (128, 256)); din("wg1b", (128, 256))
    din("wc1t", (128, 128)); din("wc1b", (128, 128))       # wc1b pre-halved
    din("wgdna", (128, 256)); din("wgdh", (128, 256))
    din("wcdna", (128, 128)); din("wcdh", (128, 128))      # wcdh pre-halved
    din("wgd1t", (128, 256)); din("wgd1b", (128, 256))
    din("wcd1t", (128, 128)); din("wcd1b", (128, 128))     # pre-halved
    # biases as [128,1] columns (gate biases pre-halved where used in ACT)
    for nm in ("bg0r", "bg0u", "cb0", "bgyr", "bgyu", "cby",
               "b1grh", "b1guh", "cb1", "bd1grh", "bd1guh", "cbd1"):
        din(nm, (128, 1))
    # attention
    din("wmem", (128, 128)); din("wq", (128, 128)); din("vcol", (128, 1))
    din("wat", (128, 128)); din("wab", (128, 128))
    # constants / masks
    din("i128", (128, 128)); din("sel2", (128, 64)); din("sel2x", (128, 128))
    din("maskadd2", (128, 16)); din("bigx", (1, SB)); din("ones1", (1, 128))
    din("ym2", (128, 16))
    din("ymk1", (1, SB), BF16)
    # vocab projection shard (per-core contents differ)
    din("w16", (128, VC), BF16); din("b16", (1, VC), BF16)

    out = nc.dram_tensor("out", [SB, VC], BF16, kind="ExternalOutput").ap()
    DBG = os.environ.get("KDEBUG", "0") == "1"
    dbg = {}
    if DBG:
        for nm in ("dbg_memT2", "dbg_mem2", "dbg_keysT2", "dbg_na"):
            dbg[nm] = nc.dram_tensor(nm, [128, SB], F32,
                                     kind="ExternalOutput").ap()
        dbg["dbg_henc"] = nc.dram_tensor("dbg_henc", [128, 128], F32,
                                         kind="ExternalOutput").ap()

    from contextlib import ExitStack
    with tile.TileContext(nc) as tc, ExitStack() as ctx:
        const = ctx.enter_context(tc.tile_pool(name="const", bufs=1))
        big = ctx.enter_context(tc.tile_pool(name="big", bufs=1))
        work = ctx.enter_context(tc.tile_pool(name="work", bufs=3))
        st = ctx.enter_context(tc.tile_pool(name="st", bufs=3))
        prep = ctx.enter_context(tc.tile_pool(name="prep", bufs=2))
        outp = ctx.enter_context(tc.tile_pool(name="outp", bufs=4))
        ps = ctx.enter_context(tc.tile_pool(name="ps", bufs=6, space="PSUM"))
        psp = ctx.enter_context(tc.tile_pool(name="psp", bufs=2, space="PSUM"))

        cst = {}

        def load(name, pool=const, dt=None):
            ap = d[name]
            t = pool.tile(list(ap.shape), dt or ap.dtype, tag=name)
            nc.sync.dma_start(t[:], ap[:])
            cst[name] = t
            return t

        for nm in ("e0gx0", "e0gx1", "e0gx2", "e0cx0", "e0cx1", "e0cx2",
                   "d0gx0", "d0gx1", "d0gx2", "d0cx0", "d0cx1", "d0cx2",
                   "wg0h", "wc0h", "wg1t", "wg1b", "wc1t", "wc1b",
                   "wgdna", "wgdh", "wcdna", "wcdh",
                   "wgd1t", "wgd1b", "wcd1t", "wcd1b",
                   "bg0r", "bg0u", "cb0", "bgyr", "bgyu", "cby",
                   "b1grh", "b1guh", "cb1", "bd1grh", "bd1guh", "cbd1",
                   "wmem", "wq", "vcol", "wat", "wab",
                   "i128", "sel2", "sel2x", "maskadd2", "bigx", "ones1",
                   "ym2", "ymk1", "w16", "b16"):
            load(nm, big if nm in ("w16", "b16") else const)

        i128 = cst["i128"]
        ones1 = cst["ones1"]

        # ---------------- x-part precomputes ----------------
        # dst[:, (t,b)] = sum_k wchunk_k[:, half].T @ src_k + bias (+BIG u-mask)
        xparts = {}
        with tc.tile_pool(name="emb", bufs=1) as embp:
            srcs = {}
            for nm in ("ex0", "ex1", "ex2", "ey0", "ey1", "ey2"):
                t = embp.tile(list(d[nm].shape), F32, tag=nm)
                nc.sync.dma_start(t[:], d[nm][:])
                srcs[nm] = t

            def xpart(dst_name, wnames, half, src_names, bias, add_big):
                dst = big.tile([128, SB], F32, tag=dst_name)
                xparts[dst_name] = dst
                for (c0, w) in _COL_CHUNKS:
                    p = psp.tile([128, 512], F32, tag="pp")
                    n = len(wnames)
                    for k in range(n):
                        wk = cst[wnames[k]]
                        g0 = half * 128
                        nc.tensor.matmul(
                            p[:, 0:w], wk[:, g0:g0 + 128],
                            srcs[src_names[k]][:, c0:c0 + w],
                            start=(k == 0), stop=(k == n - 1 and not add_big))
                    if add_big:
                        nc.tensor.matmul(
                            p[:, 0:w], ones1[:], cst["bigx"][:, c0:c0 + w],
                            start=False, stop=True, skip_group_check=True)
                    nc.scalar.activation(dst[:, c0:c0 + w], p[:, 0:w],
                                         AF.Identity, bias=cst[bias][:])

            exs = ("ex0", "ex1", "ex2"); eys = ("ey0", "ey1", "ey2")
            xpart("xg0r", ("e0gx0", "e0gx1", "e0gx2"), 0, exs, "bg0r", False)
            xpart("xg0u", ("e0gx0", "e0gx1", "e0gx2"), 1, exs, "bg0u", True)
            xpart("xc0", ("e0cx0", "e0cx1", "e0cx2"), 0, exs, "cb0", False)
            xpart("yg0r", ("d0gx0", "d0gx1", "d0gx2"), 0, eys, "bgyr", False)
            xpart("yg0u", ("d0gx0", "d0gx1", "d0gx2"), 1, eys, "bgyu", False)
            xpart("yc0", ("d0cx0", "d0cx1", "d0cx2"), 0, eys, "cby", False)

        # ---------------- GRU cell ----------------
        def gru(h, terms_r, terms_u, c_pre, wch_half, gbr=None, gbu=None,
                cb=None, htag="h"):
            G = ps.tile([128, 128], F32, tag="ps")
            for i, (l, r_) in enumerate(terms_r):
                nc.tensor.matmul(G[:, 0:64], l, r_, start=(i == 0),
                                 stop=(i == len(terms_r) - 1),
                                 skip_group_check=(i > 0))
            for i, (l, r_) in enumerate(terms_u):
                nc.tensor.matmul(G[:, 64:128], l, r_, start=(i == 0),
                                 stop=(i == len(terms_u) - 1),
                                 skip_group_check=(i > 0))
            tg = work.tile([128, 128], F32, tag="tg")
            if gbr is None:
                nc.scalar.activation(tg[:], G[:], AF.Tanh, scale=0.5)
            else:
                nc.scalar.activation(tg[:, 0:64], G[:, 0:64], AF.Tanh,
                                     bias=gbr[:], scale=0.5)
                nc.scalar.activation(tg[:, 64:128], G[:, 64:128], AF.Tanh,
                                     bias=gbu[:], scale=0.5)
            m1 = work.tile([128, 64], F32, tag="m1")
            nc.vector.tensor_mul(m1[:], tg[:, 0:64], h[:])
            rh2 = work.tile([128, 64], F32, tag="rh2")
            nc.vector.tensor_add(rh2[:], m1[:], h[:])
            C = ps.tile([128, 64], F32, tag="ps")
            cterms = list(c_pre) + [(wch_half[:], rh2[:])]
            for i, (l, r_) in enumerate(cterms):
                nc.tensor.matmul(C[:], l, r_, start=(i == 0),
                                 stop=(i == len(cterms) - 1),
                                 skip_group_check=(i > 0))
            c_sb = work.tile([128, 64], F32, tag="c")
            if cb is None:
                nc.scalar.activation(c_sb[:], C[:], AF.Tanh)
            else:
                nc.scalar.activation(c_sb[:], C[:], AF.Tanh, bias=cb[:])
            u = work.tile([128, 64], F32, tag="u")
            nc.vector.tensor_scalar(u[:], tg[:, 64:128], 0.5, 0.5, OP.mult, OP.add)
            dd = work.tile([128, 64], F32, tag="dd")
            nc.vector.tensor_sub(dd[:], h[:], c_sb[:])
            p2 = work.tile([128, 64], F32, tag="p2")
            nc.vector.tensor_mul(p2[:], u[:], dd[:])
            hn = st.tile([128, 64], F32, tag=htag)
            nc.vector.tensor_add(hn[:], p2[:], c_sb[:])
            return hn

        # ---------------- encoder ----------------
        h0 = st.tile([128, 64], F32, tag="h0")
        h1 = st.tile([128, 64], F32, tag="h1")
        nc.vector.memset(h0[:], 0.0)
        nc.vector.memset(h1[:], 0.0)
        memT2 = big.tile([128, SB], F32, tag="memT2")   # [h, (t,b)]
        mem2 = big.tile([128, SB], F32, tag="mem2")     # [(tpar,b), c*128+h]
        xg0r, xg0u, xc0 = xparts["xg0r"], xparts["xg0u"], xparts["xc0"]
        yg0r, yg0u, yc0 = xparts["yg0r"], xparts["yg0u"], xparts["yc0"]
        wg0h, wc0h = cst["wg0h"], cst["wc0h"]
        wg1t, wg1b = cst["wg1t"], cst["wg1b"]
        wc1t, wc1b = cst["wc1t"], cst["wc1b"]

        tr_ps = None
        for t in range(S):
            sl = slice(t * 64, (t + 1) * 64)
            h0 = gru(h0,
                     [(wg0h[:, 0:128], h0[:]), (i128[:], xg0r[:, sl])],
                     [(wg0h[:, 128:256], h0[:]), (i128[:], xg0u[:, sl])],
                     [(i128[:], xc0[:, sl])], wc0h, htag="h0")
            h1 = gru(h1,
                     [(wg1t[:, 0:128], h0[:]), (wg1b[:, 0:128], h1[:])],
                     [(wg1t[:, 128:256], h0[:]), (wg1b[:, 128:256], h1[:]),
                      (ones1[:], cst["bigx"][:, sl])],
                     [(wc1t[:], h0[:])], wc1b,
                     gbr=cst["b1grh"], gbu=cst["b1guh"], cb=cst["cb1"],
                     htag="h1")
            nc.vector.tensor_copy(memT2[:, sl], h1[:])
            if t % 2 == 1:
                cc = t // 2
                tr_ps = ps.tile([128, 128], F32, tag="ps")
                nc.tensor.transpose(tr_ps[:], memT2[:, cc * 128:(cc + 1) * 128],
                                    i128[:])
                nc.scalar.copy(mem2[:, cc * 128:(cc + 1) * 128], tr_ps[:])

        if DBG:
            nc.sync.dma_start(dbg["dbg_memT2"][:], memT2[:])
            nc.sync.dma_start(dbg["dbg_mem2"][:], mem2[:])
            henc = work.tile([128, 128], F32, tag="henc")
            nc.vector.tensor_copy(henc[:, 0:64], h0[:])
            nc.vector.tensor_copy(henc[:, 64:128], h1[:])
            nc.sync.dma_start(dbg["dbg_henc"][:], henc[:])

        # keysT2[u, (t,b)] = W_mem.T @ memT2
        keysT2 = big.tile([128, SB], F32, tag="keysT2")
        for (c0, w) in _COL_CHUNKS:
            kp = psp.tile([128, 512], F32, tag="pp")
            nc.tensor.matmul(kp[:, 0:w], cst["wmem"][:], memT2[:, c0:c0 + w],
                             start=True, stop=True)
            nc.scalar.copy(keysT2[:, c0:c0 + w], kp[:, 0:w])

        if DBG:
            nc.sync.dma_start(dbg["dbg_keysT2"][:], keysT2[:])
            naF = big.tile([128, SB], F32, tag="naF")

        # ---------------- decoder ----------------
        naT = st.tile([128, 64], F32, tag="na")
        nc.vector.memset(naT[:], 0.0)
        naT16 = big.tile([128, SB], BF16, tag="naT16")
        wgdna, wgdh = cst["wgdna"], cst["wgdh"]
        wcdna, wcdh = cst["wcdna"], cst["wcdh"]
        wgd1t, wgd1b = cst["wgd1t"], cst["wgd1b"]
        wcd1t, wcd1b = cst["wcd1t"], cst["wcd1b"]

        for t in range(S):
            sl = slice(t * 64, (t + 1) * 64)
            h0 = gru(h0,
                     [(wgdna[:, 0:128], naT[:]), (wgdh[:, 0:128], h0[:]),
                      (i128[:], yg0r[:, sl])],
                     [(wgdna[:, 128:256], naT[:]), (wgdh[:, 128:256], h0[:]),
                      (i128[:], yg0u[:, sl])],
                     [(wcdna[:], naT[:]), (i128[:], yc0[:, sl])],
                     wcdh, htag="h0")
            h1 = gru(h1,
                     [(wgd1t[:, 0:128], h0[:]), (wgd1b[:, 0:128], h1[:])],
                     [(wgd1t[:, 128:256], h0[:]), (wgd1b[:, 128:256], h1[:])],
                     [(wcd1t[:], h0[:])], wcd1b,
                     gbr=cst["bd1grh"], gbu=cst["bd1guh"], cb=cst["cbd1"],
                     htag="h1")

            # ---- attention ----
            qT = ps.tile([128, 64], F32, tag="ps")
            nc.tensor.matmul(qT[:], cst["wq"][:], h1[:], start=True, stop=True)
            pre = prep.tile([128, SB], F32, tag="pre")
            nc.vector.tensor_add(
                pre[:].rearrange("p (s b) -> p s b", b=64),
                keysT2[:].rearrange("p (s b) -> p s b", b=64),
                qT[:, None, :].broadcast_to((128, S, 64)))
            th = prep.tile([128, SB], F32, tag="th")
            nc.scalar.activation(th[:], pre[:], AF.Tanh)
            sc2 = ps.tile([128, 16], F32, tag="ps")
            nc.tensor.matmul(sc2[:, 0:10], i128[:], cst["maskadd2"][:, 0:10],
                             start=True, stop=False, skip_group_check=True)
            for s in range(S):
                half = s % 2
                out_ap = sc2[half * 64:(half + 1) * 64, s // 2:s // 2 + 1]
                nc.tensor.matmul(out_ap, th[:, s * 64:(s + 1) * 64],
                                 cst["vcol"][:], start=False, stop=(s >= S - 2),
                                 tile_position=(0, half * 64),
                                 skip_group_check=True)
            exp2 = work.tile([128, 16], F32, tag="exp2")
            nc.scalar.activation(exp2[:, 0:10], sc2[:, 0:10], AF.Exp)
            denp = ps.tile([128, 16], F32, tag="ps")
            nc.tensor.matmul(denp[:, 0:10], cst["sel2x"][:], exp2[:, 0:10],
                             start=True, stop=True)
            den2 = work.tile([128, 1], F32, tag="den2")
            nc.vector.reduce_sum(den2[:], denp[:, 0:10], axis=AX.X)
            rec2 = work.tile([128, 1], F32, tag="rec2")
            nc.vector.reciprocal(rec2[:], den2[:])
            prods = prep.tile([128, SB], F32, tag="prods")
            for c in range(10):
                nc.vector.tensor_scalar(prods[:, c * 128:(c + 1) * 128],
                                        mem2[:, c * 128:(c + 1) * 128],
                                        exp2[:, c:c + 1], rec2[:],
                                        OP.mult, OP.mult)
            ctxT = ps.tile([128, 64], F32, tag="ps")
            for c in range(10):
                nc.tensor.matmul(ctxT[:], prods[:, c * 128:(c + 1) * 128],
                                 cst["sel2"][:], start=(c == 0), stop=(c == 9))
            ctx_sb = work.tile([128, 64], F32, tag="ctx")
            nc.vector.tensor_copy(ctx_sb[:], ctxT[:])
            naP = ps.tile([128, 64], F32, tag="ps")
            nc.tensor.matmul(naP[:], cst["wat"][:], h1[:], start=True, stop=False)
            nc.tensor.matmul(naP[:], cst["wab"][:], ctx_sb[:], start=False,
                             stop=True)
            naT = st.tile([128, 64], F32, tag="na")
            nc.scalar.copy(naT[:], naP[:])
            nc.vector.tensor_copy(naT16[:, sl], naP[:])
            if DBG:
                nc.vector.tensor_copy(naF[:, sl], naP[:])

            # ---- vocab projection for the completed row-chunk ----
            if t % 2 == 1:
                rc = t // 2
                rsl = slice(rc * 128, (rc + 1) * 128)
                for ccv in range(VC // 512):
                    csl = slice(ccv * 512, (ccv + 1) * 512)
                    pp = psp.tile([128, 512], F32, tag="pp")
                    nc.tensor.matmul(pp[:], naT16[:, rsl], cst["w16"][:, csl],
                                     start=True, stop=False)
                    nc.tensor.matmul(pp[:], cst["ymk1"][:, rsl],
                                     cst["b16"][:, csl], start=False, stop=True,
                                     skip_group_check=True)
                    ot = outp.tile([128, 512], BF16, tag="ot")
                    if ccv % 2 == 0:
                        nc.scalar.mul(ot[:], pp[:], cst["ym2"][:, rc:rc + 1])
                    else:
                        nc.vector.tensor_scalar_mul(ot[:], pp[:],
                                                    cst["ym2"][:, rc:rc + 1])
                    nc.sync.dma_start(out[rsl, csl], ot[:])

        if DBG:
            nc.sync.dma_start(dbg["dbg_na"][:], naF[:])

    nc.compile()
    return nc


_NC_CACHE = None


def _get_nc():
    global _NC_CACHE
    if _NC_CACHE is None:
        _NC_CACHE = _build_nc()
    return _NC_CACHE


def _host_prep(inp):
    f32 = np.float32
    x = np.asarray(inp["x"]); y = np.asarray(inp["y"])
    xl = np.asarray(inp["x_length"]); yl = np.asarray(inp["y_length"])
    emb = np.asarray(inp["embedding"], f32)
    g = lambda k: np.asarray(inp[k], f32)

    ex = emb[x]                       # [B,S,E]
    ey = emb[y]
    exT = np.ascontiguousarray(ex.transpose(2, 1, 0).reshape(E, SB))
    eyT = np.ascontiguousarray(ey.transpose(2, 1, 0).reshape(E, SB))
    x_valid = (np.arange(S)[None, :] < xl[:, None])   # [B,S]
    y_valid = (np.arange(S)[None, :] < yl[:, None])

    m = {}
    m["ex0"], m["ex1"], m["ex2"] = exT[0:128], exT[128:256], exT[256:300]
    m["ey0"], m["ey1"], m["ey2"] = eyT[0:128], eyT[128:256], eyT[256:300]

    e0_gk, e0_ck = g("e0_gk"), g("e0_ck")
    d0_gk, d0_ck = g("d0_gk"), g("d0_ck")
    for nm, w in (("e0gx", e0_gk), ("e0cx", e0_ck),
                  ("d0gx", d0_gk), ("d0cx", d0_ck)):
        m[f"{nm}0"], m[f"{nm}1"], m[f"{nm}2"] = w[0:128], w[128:256], w[256:300]
    m["wg0h"] = e0_gk[300:428]
    m["wc0h"] = 0.5 * e0_ck[300:428]
    e1_gk, e1_ck = g("e1_gk"), g("e1_ck")
    m["wg1t"], m["wg1b"] = e1_gk[0:128], e1_gk[128:256]
    m["wc1t"], m["wc1b"] = e1_ck[0:128], 0.5 * e1_ck[128:256]
    m["wgdna"], m["wgdh"] = d0_gk[300:428], d0_gk[428:556]
    m["wcdna"], m["wcdh"] = d0_ck[300:428], 0.5 * d0_ck[428:556]
    d1_gk, d1_ck = g("d1_gk"), g("d1_ck")
    m["wgd1t"], m["wgd1b"] = d1_gk[0:128], d1_gk[128:256]
    m["wcd1t"], m["wcd1b"] = d1_ck[0:128], 0.5 * d1_ck[128:256]

    col = lambda v: np.ascontiguousarray(v.reshape(128, 1))
    e0_gb, e1_gb = g("e0_gb"), g("e1_gb")
    d0_gb, d1_gb = g("d0_gb"), g("d1_gb")
    m["bg0r"], m["bg0u"], m["cb0"] = col(e0_gb[0:128]), col(e0_gb[128:256]), col(g("e0_cb"))
    m["bgyr"], m["bgyu"], m["cby"] = col(d0_gb[0:128]), col(d0_gb[128:256]), col(g("d0_cb"))
    m["b1grh"], m["b1guh"], m["cb1"] = col(0.5 * e1_gb[0:128]), col(0.5 * e1_gb[128:256]), col(g("e1_cb"))
    m["bd1grh"], m["bd1guh"], m["cbd1"] = col(0.5 * d1_gb[0:128]), col(0.5 * d1_gb[128:256]), col(g("d1_cb"))

    m["wmem"], m["wq"] = g("W_mem"), g("W_q")
    m["vcol"] = col(g("v_att"))
    W_attn = g("W_attn")
    m["wat"], m["wab"] = W_attn[0:128], W_attn[128:256]

    m["i128"] = np.eye(128, dtype=f32)
    p = np.arange(128)
    sel2 = np.zeros((128, 64), f32); sel2[p, p % 64] = 1.0
    m["sel2"] = sel2
    m["sel2x"] = (p[:, None] % 64 == p[None, :] % 64).astype(f32)
    mm_ = np.where(x_valid, 0.0, NEG).astype(f32).T          # [s, b]
    mk2 = np.zeros((128, 16), f32)
    mk2[:, 0:10] = mm_.reshape(10, 2, 64).transpose(1, 2, 0).reshape(128, 10)
    m["maskadd2"] = mk2
    m["bigx"] = np.ascontiguousarray(
        (BIG * (~x_valid).T.astype(f32)).reshape(1, SB))
    m["ones1"] = np.ones((1, 128), f32)
    ymrow = np.ascontiguousarray(y_valid.T.astype(f32).reshape(SB))  # r=t*64+b
    ym2 = np.zeros((128, 16), f32)
    ym2[:, 0:10] = ymrow.reshape(10, 128).T
    m["ym2"] = ym2
    m["ymk1"] = ymrow.reshape(1, SB).astype(NPBF16)

    in_maps = []
    W_proj = g("W_proj"); b_proj = g("b_proj")
    Wfull = np.zeros((128, NCORES * VSH), f32); Wfull[:, :V] = W_proj
    bfull = np.zeros(NCORES * VSH, f32); bfull[:V] = b_proj
    for k in range(NCORES):
        wk = np.zeros((128, VC), f32)
        wk[:, :VSH] = Wfull[:, k * VSH:(k + 1) * VSH]
        bk = np.zeros((1, VC), f32)
        bk[0, :VSH] = bfull[k * VSH:(k + 1) * VSH]
        mk = dict(m)
        mk["w16"] = wk.astype(NPBF16)
        mk["b16"] = bk.astype(NPBF16)
        in_maps.append(mk)
    return in_maps


def kernel(**inputs):
    global LAST_RESULT
    nc = _get_nc()
    in_maps = _host_prep(inputs)
    res = run_bass_kernel_spmd(nc, in_maps, list(range(NCORES)))
    LAST_RESULT = res
    shards = [np.asarray(r["out"], dtype=NPBF16).astype(np.float32)[:, :VSH]
              for r in res.results]
    full = np.concatenate(shards, axis=1)[:, :V]          # [1280, V]
    return np.ascontiguousarray(
        full.reshape(S, B, V).transpose(1, 0, 2))         # [B, S, V]


if __name__ == "__main__":
    # smoke build
    _get_nc()
    print("built ok")

